# revision 31
# baseline (speedup 1.0000x reference)
"""DeltaSynapse kernel for Trainium2 (8 NeuronCores, SPMD).

Reference computation:
    Xpre[b,e,o] = sum_d delaymap[d,e,o] * Xd[d,b,e]
    I[b,o]      = sum_e (signs*W)[e,o] * Xpre[b,e,o]

Folded:  I[b,o] = sum_{d,e} (delaymap[d,e,o] * Weff[e,o]) * Xd[d,b,e]
i.e. a sum of D=8 matmuls I += Xd[d] @ (delaymap[d] . Weff).

delaymap is a one-hot over the 8 delays -- pure structure, 3 bits per
synapse -- which the baseline streamed as 128 MiB of fp32. Compressed
here into bit-planes of didx = argmax_d delaymap (hi = didx>>2 picks
the plane pair, l0 = didx&1 ships as its own 0/1 plane, l1 =
(didx>>1)&1 rides the SIGN bit):

    M     = |Weff| masked by (hi==a)      (row signs of Weff moved to
    Om_a  = M * (1-2*l1)                   the X side, exact for +-1)

Derived on-device per o-range (DVE, all exact since l-bits only flip
signs / zero halves):  tau_a = Om_a . l0  (one fused 2x-mode mult),
s_a = relu(-Om_a), t_a = relu(-tau_a)  (two fused tensor_scalar ops).
Using M = Om + 2*relu(-Om), M.l0 = tau + 2*relu(-tau), the one-hot
interpolation becomes 8 matmuls with host-precomputed X combos:

  I = sum_a [  Xd[4a]              @ Om_a
             + (Xd[4a+1]-Xd[4a])   @ tau_a
             + (Xd[4a]+Xd[4a+2])   @ s_a
             + (Xd[4a+1]-Xd[4a]+Xd[4a+3]-Xd[4a+2]) @ t_a ]

HBM traffic per core: 3 fp16 planes e-sliced (~3.15 MiB) + tiny X
combos, vs 18 MiB baseline.

Schedule (trace-calibrated on this part):
  - PE at 2.4 GHz gives spacing N/2.4+2.5 ns per matmul -- but only
    after ~3.5us of sustained work (DVFS). ~24 warm-up matmuls on a
    memset tile burn the 1.2 GHz phase while the first planes stream;
    a second filler batch bridges the one unavoidable stream wait
    (wl1's SWDGE descriptor-gen chain) so the clock never drops.
  - SWDGE descriptor-gen costs ~1us per dma_start on Pool: 6 range
    tensors + nothing else on the stream queue; yc rides HWDGE and
    lands before the SWDGE stream ramps (HWDGE starves at ~10 GB/s
    once SWDGE streams, so only outputs use it afterwards).
  - Products run on DVE only (Pool tensor ops contend 3x with DVE on
    SBUF), ordered s -> tau -> t; matmuls consume Om directly, then
    s, tau, t, so the earliest-ready planes are used first.
  - Ranges staircase 128->512->128: early PE start, short final tail.
  - Two tile pools (SBUF+PSUM): each extra pool costs a multi-engine
    barrier cascade at kernel end (~0.4us apiece).

Sharding: contraction (pre-neuron e) dim across 8 cores, 256 rows
each; every core emits a full [16, 2048] partial, host sums.
"""

import numpy as np

D, B, N = 8, 16, 2048
NCORES = 8
P = 128                 # SBUF partitions / matmul contraction tile
ESH = N // NCORES       # per-core pre-dim shard = 256
ECH = ESH // P          # e-chunks per core = 2
O_WIDTHS = [128, 384, 512, 512, 384, 128]
O_RANGES = []
_o = 0
for _w in O_WIDTHS:
    O_RANGES.append((_o, _o + _w))
    _o += _w
assert _o == N
NR = len(O_RANGES)
NWARM = 18              # PE warm-up matmuls (burn the DVFS mid-clock)
NFILL = {0: 16}         # filler matmuls after range r: bridge stream
                        # waits at full clock (idle PE re-throttles)

_prog_cache = {}


def _build_program():
    from concourse import bacc, tile
    from concourse import mybir

    f32 = mybir.dt.float32
    f16 = mybir.dt.float16
    MULT = mybir.AluOpType.mult
    MAX = mybir.AluOpType.max

    nc = bacc.Bacc(enable_partition_id=False)
    # Host-prepared fp16 layouts (see _shard_inputs):
    #   wl{r}: [P, ECH, 3, w_r]  planes (Om0, Om1, l0), o-range r
    #   yc   : [P, ECH, 8, B]    X-side combos (matmul lhsT order)
    wls = {}
    for r, (o0, o1) in enumerate(O_RANGES):
        wls[r] = nc.dram_tensor(f"wl{r}", [P, ECH, 3, o1 - o0], f16,
                                kind="ExternalInput")
    ycd = nc.dram_tensor("yc", [P, ECH, 8, B], f16, kind="ExternalInput")
    out = nc.dram_tensor("out", [B, N], f32, kind="ExternalOutput")
    wout = nc.dram_tensor("wout", [B, 1], f32, kind="ExternalOutput")

    with tile.TileContext(nc) as tc:
        with (
            tc.tile_pool(name="sb", bufs=3) as sbp,
            tc.tile_pool(name="psum", bufs=7, space="PSUM") as ppool,
        ):
            yc = sbp.tile([P, ECH, 8, B], f16)
            warm_sb = sbp.tile([P, 256], f16)
            warm_o = sbp.tile([B, 1], f32)
            wl_tiles = {}
            for r, (o0, o1) in enumerate(O_RANGES):
                wl_tiles[r] = sbp.tile([P, ECH, 3, o1 - o0], f16,
                                       tag="wl", name=f"wl{r}")

            # yc and the small wl0 ride the two HWDGE queues (sync +
            # scalar) in parallel -- both land before the SWDGE stream
            # ramps at ~9.6us and starves them. The SWDGE descriptor-gen
            # chain (1us/dma on Pool) then starts directly on wl1,
            # pulling every range's completion semaphore ~1us earlier.
            nc.sync.dma_start(yc[:], ycd[:])
            nc.scalar.dma_start(wl_tiles[0][:], wls[0][:])
            for r in range(1, NR):
                nc.gpsimd.dma_start(wl_tiles[r][:], wls[r][:])

            # PE DVFS warm-up on a zero tile (see module docstring).
            warm_ps = ppool.tile([B, 512], f32, tag="ps", name="warmps")
            nc.vector.memset(warm_sb[:], 0.0)
            for i in range(NWARM):
                nc.tensor.matmul(warm_ps[:, :128], warm_sb[:, :B],
                                 warm_sb[:, B:B + 128],
                                 start=(i == 0), stop=(i == NWARM - 1))
            nc.scalar.copy(warm_o[:], warm_ps[:, :1])

            for r, (o0, o1) in enumerate(O_RANGES):
                w = o1 - o0
                psum = ppool.tile([B, 512], f32, tag="ps", name=f"ps{r}")
                wl = wl_tiles[r]
                wd = sbp.tile([P, ECH, 6, 512], f16, tag="wd")
                lam = wl[:, :, 2, :].unsqueeze(2).broadcast_to([P, ECH, 2, w])
                # derived planes in readiness order: s, tau, t
                nc.vector.tensor_scalar(wd[:, :, 2:4, :w], wl[:, :, 0:2, :],
                                        -1.0, 0.0, MULT, MAX)       # s
                nc.vector.tensor_mul(wd[:, :, 0:2, :w], wl[:, :, 0:2, :],
                                     lam)                           # tau
                nc.vector.tensor_scalar(wd[:, :, 4:6, :w],
                                        wd[:, :, 0:2, :w],
                                        -1.0, 0.0, MULT, MAX)       # t
                # matmuls: direct Om planes first, then s, tau, t
                plane_list = [
                    (0, lambda c: wl[:, c, 0, :]),
                    (1, lambda c: wl[:, c, 1, :]),
                    (4, lambda c: wd[:, c, 2, :w]),
                    (5, lambda c: wd[:, c, 3, :w]),
                    (2, lambda c: wd[:, c, 0, :w]),
                    (3, lambda c: wd[:, c, 1, :w]),
                    (6, lambda c: wd[:, c, 4, :w]),
                    (7, lambda c: wd[:, c, 5, :w]),
                ]
                first = True
                nmm = 0
                for j, rhsf in plane_list:
                    for c in range(ECH):
                        nmm += 1
                        nc.tensor.matmul(psum[:, :w], yc[:, c, j, :],
                                         rhsf(c),
                                         start=first,
                                         stop=(nmm == 2 * len(plane_list)))
                        first = False
                if r in NFILL:
                    # keep PE busy at speed through the known stream wait
                    fp = ppool.tile([B, 512], f32, tag="ps", name=f"fp{r}")
                    nf = NFILL[r]
                    for i in range(nf):
                        nc.tensor.matmul(fp[:, :128], warm_sb[:, :B],
                                         warm_sb[:, B:B + 128],
                                         start=(i == 0), stop=(i == nf - 1))
                    nc.scalar.copy(warm_o[:], fp[:, :1])
                    if r == max(NFILL):
                        nc.sync.dma_start(wout[:], warm_o[:])
                # o-range complete: stream it out immediately
                out_t = sbp.tile([B, 512], f32, tag="out", name=f"o{r}")
                nc.scalar.copy(out_t[:, :w], psum[:, :w])
                nc.sync.dma_start(out[:, o0:o1], out_t[:, :w])

    nc.compile()
    return nc


def _get_program():
    if "nc" not in _prog_cache:
        _prog_cache["nc"] = _build_program()
    return _prog_cache["nc"]


def _shard_inputs(Xd, delaymap, W, signs):
    """Compress delaymap to sign-packed bit-planes; per-core fp16 maps."""
    Xd = np.asarray(Xd, dtype=np.float32)
    delaymap = np.asarray(delaymap, dtype=np.float32)
    W = np.asarray(W, dtype=np.float32)
    signs = np.asarray(signs, dtype=np.float32)

    didx = np.argmax(delaymap, axis=0).astype(np.uint8)     # (N, N) in [0,8)
    Weff = signs * W
    # per-row (pre-neuron) sign, exact on Weff's support
    rs = np.sign(np.sum(signs, axis=1)).astype(np.float32)  # (N,)
    Wmag = np.abs(Weff)
    hi = didx >> 2
    sg1 = 1.0 - 2.0 * ((didx >> 1) & 1).astype(np.float32)  # l1 -> sign
    planes = np.empty((3, N, N), dtype=np.float16)
    M0 = np.where(hi == 0, Wmag, 0.0)
    planes[0] = M0 * sg1                                    # Om0
    planes[1] = (Wmag - M0) * sg1                           # Om1
    planes[2] = (didx & 1).astype(np.float16)               # l0

    in_maps = []
    for k in range(NCORES):
        esl = slice(k * ESH, (k + 1) * ESH)
        # [3, ESH, N] -> [P, ECH, 3, N] (e = c*128 + p), then o-range slices
        pl = planes[:, esl, :].reshape(3, ECH, P, N).transpose(2, 1, 0, 3)
        m = {}
        for r, (o0, o1) in enumerate(O_RANGES):
            m[f"wl{r}"] = np.ascontiguousarray(pl[:, :, :, o0:o1])
        # X-side combos (row signs folded in), lhsT order j:
        #   [X0, X4, X1-X0, X5-X4, X0+X2, X4+X6,
        #    X1-X0+X3-X2, X5-X4+X7-X6]
        xe = Xd[:, :, esl] * rs[esl][None, None, :]         # (D, B, ESH)
        Y = np.empty((8, B, ESH), dtype=np.float32)
        for a in (0, 1):
            b4 = xe[4 * a:4 * a + 4]
            Y[0 + a] = b4[0]
            Y[2 + a] = b4[1] - b4[0]
            Y[4 + a] = b4[0] + b4[2]
            Y[6 + a] = b4[1] - b4[0] + b4[3] - b4[2]
        m["yc"] = np.ascontiguousarray(
            Y.reshape(8, B, ECH, P).transpose(3, 2, 0, 1).astype(np.float16)
        )
        in_maps.append(m)
    return in_maps


def _run(in_maps, trace=False, **kw):
    from concourse.bass_utils import run_bass_kernel_spmd

    nc = _get_program()
    return run_bass_kernel_spmd(nc, in_maps, list(range(NCORES)), trace=trace, **kw)


def _gather(res):
    acc = np.zeros((B, N), dtype=np.float64)
    for k in range(NCORES):
        acc += res.results[k]["out"].astype(np.float64)
    return acc.astype(np.float32)


def kernel(Xd, X, delaymap, W, signs):
    in_maps = _shard_inputs(Xd, delaymap, W, signs)
    return _gather(_run(in_maps))
